# revision 5
# baseline (speedup 1.0000x reference)
"""Trainium2 Bass kernel for the KG hetero GNN (2-layer GATv2, 7 relations).

Sharding: layer 1 is computed fully on every core (v0); layer 2 only needs
relations into 'news' (the reference returns x['news'] @ lin_w, so x3 of the
other node types is dead) and is sharded by news row range across 8 cores.

Edge aggregation: edges sorted by dst, packed into "macros" of C*128 edge
slots with <=127 distinct dsts (slot 127 reserved for padding/trash).
Per chunk: indirect-gather xl[src], xr[dst] rows; t = lrelu(xl+xr);
e = sum(t*att); w = exp(e)  (the segment-max shift of softmax is skipped:
|e| stays orders of magnitude below the fp32 exp overflow threshold, and
the normalized ratio is identical up to rounding); a one-hot matmul
(lhsT[edge,slot] = w * (slot==dst_slot)) accumulates [slots, H+1] into PSUM
with column H accumulating w itself (softmax denominator); each macro is
flushed with an indirect scatter-add (CCE add) into a per-relation
accumulator table [rows, H+1].  A normalize pass divides by the
denominator, sums relations, adds bias, applies relu, and writes the
result transposed (x2T) so the next layer's projections need no on-device
transposes.
"""

import sys

sys.path.insert(0, "/opt/trn_rl_repo")

import numpy as np

import concourse.bass as bass
import concourse.mybir as mybir
import concourse.tile as tile
from concourse import bacc
from concourse.bass import ds
from concourse.bass_utils import run_bass_kernel_spmd
from concourse.masks import make_identity

AF = mybir.ActivationFunctionType
ALU = mybir.AluOpType
F32 = mybir.dt.float32
I32 = mybir.dt.int32
P = 128

N_CORES = 8
RELS = [('ne', 'news', 'entities'), ('en', 'entities', 'news'),
        ('nt', 'news', 'topic'), ('tn', 'topic', 'news'),
        ('nk', 'news', 'kg'), ('kn', 'kg', 'news'),
        ('ee', 'entities', 'entities')]
L2_RELS = [('en', 'entities'), ('tn', 'topic'), ('kn', 'kg')]
FEAT = {'news': 768, 'entities': 256, 'topic': 128, 'kg': 256}
NNODE = {'news': 100000, 'entities': 200000, 'topic': 500, 'kg': 200000}
NPAD = {'news': 100352, 'entities': 200704, 'topic': 512, 'kg': 200704}
H1, H2 = 256, 128
NEWS_SLICE = NPAD['news'] // N_CORES        # 12544
C = 2                                        # chunks per macro
TRASH = 127                                  # reserved slot


# ---------------------------------------------------------------- host prep

def _build_macros(src, dst, trash_row):
    """Sort edges by dst; pack into macros of C*128 edge slots with <=127
    distinct dsts.  Returns [nmac,C,128] src/dst/slot and [nmac,128] rows."""
    order = np.argsort(dst, kind='stable')
    src, dst = np.asarray(src)[order], np.asarray(dst)[order]
    E = len(src)
    cap = C * P
    src_m, dst_m, slot_m, rows_m = [], [], [], []
    i = 0
    while i < E:
        j = min(i + cap, E)
        d = dst[i:j]
        uniq = np.unique(d)
        if len(uniq) > TRASH:
            cut = int(np.searchsorted(d, uniq[TRASH]))
            j = i + cut
            d = dst[i:j]
            uniq = np.unique(d)
        ns = j - i
        s_pad = np.zeros(cap, np.int64)
        s_pad[:ns] = src[i:j]
        d_pad = np.zeros(cap, np.int64)
        d_pad[:ns] = d
        sl_pad = np.full(cap, TRASH, np.int64)
        sl_pad[:ns] = np.searchsorted(uniq, d)
        rows = np.full(P, trash_row, np.int64)
        rows[:len(uniq)] = uniq
        src_m.append(s_pad)
        dst_m.append(d_pad)
        slot_m.append(sl_pad)
        rows_m.append(rows)
        i = j
    nmac = len(src_m)
    out = (np.array(src_m, np.int32).reshape(nmac, C, P),
           np.array(dst_m, np.int32).reshape(nmac, C, P),
           np.array(slot_m, np.float32).reshape(nmac, C, P),
           np.array(rows_m, np.int32).reshape(nmac, P))
    # interleave macro order so consecutive flushes touch different dst
    # regions (scatter-add descriptors of in-flight DMAs must not collide)
    if nmac > 16:
        stride = max(1, nmac // 16)
        perm = np.concatenate([np.arange(k, nmac, stride) for k in range(stride)])
        out = tuple(a[perm] for a in out)
    return out


def host_prep(inputs):
    params = inputs['params']
    x = {t: np.asarray(inputs['x_' + t], np.float32) for t in FEAT}

    full_edges = {}
    for name, s, d in RELS:
        e = np.asarray(inputs['e_' + name])
        n_sl = min(NNODE[s], NNODE[d])
        loops = np.arange(n_sl, dtype=np.int64)
        full_edges[name] = (np.concatenate([e[0].astype(np.int64), loops]),
                            np.concatenate([e[1].astype(np.int64), loops]))

    l1p = params['layers'][0]
    l2p = params['layers'][1]
    spec = {'l1': {}, 'l2': {}}
    common = {}

    for t in FEAT:
        xt = np.zeros((FEAT[t], NPAD[t]), np.float32)
        xt[:, :NNODE[t]] = np.asarray(x[t]).T
        common['xT_' + t] = xt

    btype = {t: np.zeros(H1, np.float32) for t in FEAT}
    for name, s, d in RELS:
        lp = l1p[name]
        common[f'Wl_{name}'] = np.asarray(lp['Wl'], np.float32)
        common[f'Wr_{name}'] = np.asarray(lp['Wr'], np.float32)
        common[f'att_{name}'] = np.asarray(lp['att'], np.float32)[None, :]
        btype[d] = btype[d] + np.asarray(lp['b'], np.float32)
        src, dst = full_edges[name]
        sm, dm, slm, rm = _build_macros(src, dst, NPAD[d])
        common[f'src1_{name}'] = sm
        common[f'xri1_{name}'] = dm
        common[f'slot1_{name}'] = slm
        common[f'rows1_{name}'] = rm
        spec['l1'][name] = sm.shape[0]
    for t in FEAT:
        common[f'B1_{t}'] = np.broadcast_to(btype[t], (P, H1)).copy()

    b2 = np.zeros(H2, np.float32)
    for name, s in L2_RELS:
        lp = l2p[name]
        b2 += np.asarray(lp['b'], np.float32)
        common[f'W2l_{name}'] = np.asarray(lp['Wl'], np.float32)
        common[f'W2r_{name}'] = np.asarray(lp['Wr'], np.float32)
        common[f'att2_{name}'] = np.asarray(lp['att'], np.float32)[None, :]
    common['B2_news'] = np.broadcast_to(b2, (P, H2)).copy()
    common['lin_w'] = np.asarray(params['lin_w'], np.float32)
    common['lin_b'] = np.broadcast_to(np.asarray(params['lin_b'], np.float32),
                                      (P, 2)).copy()

    per_core = [dict(common) for _ in range(N_CORES)]
    for name, s in L2_RELS:
        src, dst = full_edges[name]
        data = []
        for k in range(N_CORES):
            lo = k * NEWS_SLICE
            m = (dst >= lo) & (dst < lo + NEWS_SLICE)
            sm, dm, slm, rm = _build_macros(src[m], dst[m] - lo, NEWS_SLICE)
            data.append((sm, dm + lo, slm, rm))   # xr idx global, rows local
        nmax = max(a[0].shape[0] for a in data)
        spec['l2'][name] = nmax
        for k in range(N_CORES):
            sm, dm, slm, rm = data[k]
            n = sm.shape[0]
            sm2 = np.zeros((nmax, C, P), np.int32)
            dm2 = np.zeros((nmax, C, P), np.int32)
            slm2 = np.full((nmax, C, P), TRASH, np.float32)
            rm2 = np.full((nmax, P), NEWS_SLICE, np.int32)
            sm2[:n], dm2[:n], slm2[:n], rm2[:n] = sm, dm, slm, rm
            pc = per_core[k]
            pc[f'src2_{name}'] = sm2
            pc[f'xri2_{name}'] = dm2
            pc[f'slot2_{name}'] = slm2
            pc[f'rows2_{name}'] = rm2
    return per_core, spec


# ---------------------------------------------------------------- device

def build_kernel(spec):
    nc = bacc.Bacc(None, target_bir_lowering=False, debug=False,
                   num_devices=N_CORES)
    inp = {}

    def ext_in(nm, shape, dt=F32):
        inp[nm] = nc.dram_tensor(nm, list(shape), dt, kind="ExternalInput")

    for t in FEAT:
        ext_in('xT_' + t, [FEAT[t], NPAD[t]])
        ext_in('B1_' + t, [P, H1])
    for name, s, d in RELS:
        ext_in(f'Wl_{name}', [FEAT[s], H1])
        ext_in(f'Wr_{name}', [FEAT[d], H1])
        ext_in(f'att_{name}', [1, H1])
        nm = spec['l1'][name]
        ext_in(f'src1_{name}', [nm, C, P], I32)
        ext_in(f'xri1_{name}', [nm, C, P], I32)
        ext_in(f'slot1_{name}', [nm, C, P])
        ext_in(f'rows1_{name}', [nm, P], I32)
    for name, s in L2_RELS:
        ext_in(f'W2l_{name}', [H1, H2])
        ext_in(f'W2r_{name}', [H1, H2])
        ext_in(f'att2_{name}', [1, H2])
        nm = spec['l2'][name]
        ext_in(f'src2_{name}', [nm, C, P], I32)
        ext_in(f'xri2_{name}', [nm, C, P], I32)
        ext_in(f'slot2_{name}', [nm, C, P])
        ext_in(f'rows2_{name}', [nm, P], I32)
    ext_in('B2_news', [P, H2])
    ext_in('lin_w', [H2, 2])
    ext_in('lin_b', [P, 2])
    out = nc.dram_tensor("out", [NEWS_SLICE, 2], F32, kind="ExternalOutput")

    intern = {}

    def itab(nm, shape):
        intern[nm] = nc.dram_tensor(nm, list(shape), F32, kind="Internal")

    def r1024(n):
        return ((n + 1023) // 1024) * 1024

    for name, s, d in RELS:
        itab(f'xl_{name}', [NPAD[s], H1])
        itab(f'xr_{name}', [NPAD[d], H1])
        itab(f'ou1_{name}', [r1024(NPAD[d] + P), H1 + 1])
    for t in FEAT:
        itab(f'x2T_{t}', [H1, NPAD[t]])
    for name, s in L2_RELS:
        itab(f'xl2_{name}', [NPAD[s], H2])
        itab(f'xr2_{name}', [NPAD['news'], H2])
        itab(f'ou2_{name}', [r1024(NEWS_SLICE + P), H2 + 1])

    with tile.TileContext(nc) as tc:
        with (
            tc.tile_pool(name="const", bufs=1) as cpool,
            tc.tile_pool(name="work", bufs=4) as work,
            tc.tile_pool(name="psA", bufs=2, space="PSUM") as psA,
            tc.tile_pool(name="psB", bufs=2, space="PSUM") as psB,
            tc.tile_pool(name="psC", bufs=1, space="PSUM") as psC,
        ):
            cst = {}
            iota = cpool.tile([P, P], I32)
            nc.gpsimd.iota(iota[:], pattern=[[1, P]], base=0, channel_multiplier=0)
            iota_f = cpool.tile([P, P], F32)
            nc.vector.tensor_copy(iota_f[:], iota[:])
            ones_col = cpool.tile([1, P], F32)
            nc.vector.memset(ones_col[:], 1.0)
            zero_t = cpool.tile([P, H1 + 1], F32)
            nc.vector.memset(zero_t[:], 0.0)
            ident = cpool.tile([P, P], F32)
            make_identity(nc, ident[:])

            def bcast_row(nm, width):
                row = cpool.tile([1, width], F32, tag=nm + "_r")
                nc.sync.dma_start(row[:], inp[nm][:])
                ps = psC.tile([P, width], F32, tag="bc")
                nc.tensor.matmul(ps[:], lhsT=ones_col[:], rhs=row[:],
                                 start=True, stop=True)
                tl = cpool.tile([P, width], F32, tag=nm + "_b")
                nc.scalar.copy(tl[:], ps[:])
                return tl

            for name, s, d in RELS:
                cst['att_' + name] = bcast_row(f'att_{name}', H1)
            for name, s in L2_RELS:
                cst['att2_' + name] = bcast_row(f'att2_{name}', H2)

            def load_w(nm, F, Hout):
                wt = cpool.tile([P, F // P, Hout], F32, tag=nm)
                nc.sync.dma_start(wt[:], inp[nm][:].rearrange("(k p) h -> p k h", p=P))
                cst[nm] = wt

            # ------------ projections helper
            def project(xT_ap, w_names, out_tables, n_rows, F, Hout):
                kch = F // P
                with tc.For_i(0, n_rows, P) as r0:
                    xt = work.tile([P, kch, P], F32, tag="xt")
                    nc.sync.dma_start(
                        xt[:],
                        xT_ap.rearrange("(k p) n -> p k n", p=P)[:, :, ds(r0, P)])
                    for i, wn in enumerate(w_names):
                        accw = psA.tile([P, Hout], F32, tag="accw")
                        for kc in range(kch):
                            nc.tensor.matmul(accw[:], lhsT=xt[:, kc, :],
                                             rhs=cst[wn][:, kc, :],
                                             start=(kc == 0), stop=(kc == kch - 1))
                        ot = work.tile([P, Hout], F32, tag="ot")
                        nc.scalar.copy(ot[:], accw[:])
                        nc.sync.dma_start(out_tables[i][ds(r0, P)], ot[:])

            # ------------ edge phase helper
            def edge_phase(nmac, H, xl_tab, xr_tab, ou_tab, att_b,
                           srcA, xriA, slotA, rowsA):
                with tc.For_i(0, nmac, 1) as m:
                    sidx = work.tile([P, C], I32, tag="sidx")
                    ridx = work.tile([P, C], I32, tag="ridx")
                    slot = work.tile([P, C], F32, tag="slot")
                    srow = work.tile([P, 1], I32, tag="srow")
                    nc.sync.dma_start(sidx[:], srcA[ds(m, 1)].rearrange("o c p -> (o p) c"))
                    nc.sync.dma_start(ridx[:], xriA[ds(m, 1)].rearrange("o c p -> (o p) c"))
                    nc.sync.dma_start(slot[:], slotA[ds(m, 1)].rearrange("o c p -> (o p) c"))
                    nc.sync.dma_start(srow[:], rowsA[ds(m, 1)].rearrange("o (p u) -> (o p) u", u=1))

                    g_xl = work.tile([P, C, H + 1], F32, tag="gxl")
                    g_xr = work.tile([P, C, H], F32, tag="gxr")
                    for c in range(C):
                        nc.gpsimd.indirect_dma_start(
                            out=g_xl[:, c, 0:H], out_offset=None, in_=xl_tab[:],
                            in_offset=bass.IndirectOffsetOnAxis(ap=sidx[:, c:c + 1], axis=0))
                        nc.gpsimd.indirect_dma_start(
                            out=g_xr[:, c, :], out_offset=None, in_=xr_tab[:],
                            in_offset=bass.IndirectOffsetOnAxis(ap=ridx[:, c:c + 1], axis=0))
                    nc.vector.memset(g_xl[:, :, H:H + 1], 1.0)

                    t = work.tile([P, C, H], F32, tag="t")
                    nc.vector.tensor_tensor(out=t[:], in0=g_xl[:, :, 0:H],
                                            in1=g_xr[:], op=ALU.add)
                    nc.scalar.activation(t[:], t[:], AF.Prelu, alpha=0.2)
                    nc.vector.tensor_tensor(
                        out=t[:], in0=t[:],
                        in1=att_b[:].rearrange("p (u d) -> p u d", u=1).to_broadcast([P, C, H]),
                        op=ALU.mult)
                    e = work.tile([P, C], F32, tag="e")
                    nc.vector.tensor_reduce(out=e[:], in_=t[:],
                                            axis=mybir.AxisListType.X, op=ALU.add)
                    w = work.tile([P, C], F32, tag="w")
                    nc.scalar.activation(w[:], e[:], AF.Exp)

                    acc = psA.tile([P, H + 1], F32, tag="acc")
                    for c in range(C):
                        S = work.tile([P, P], F32, tag="S")
                        nc.vector.tensor_scalar(out=S[:], in0=iota_f[:],
                                                scalar1=slot[:, c:c + 1],
                                                scalar2=w[:, c:c + 1],
                                                op0=ALU.is_equal, op1=ALU.mult)
                        nc.tensor.matmul(acc[:], lhsT=S[:], rhs=g_xl[:, c, :],
                                         start=(c == 0), stop=(c == C - 1))
                    flush = work.tile([P, H + 1], F32, tag="flush")
                    nc.scalar.copy(flush[:], acc[:])
                    nc.gpsimd.indirect_dma_start(
                        out=ou_tab[:],
                        out_offset=bass.IndirectOffsetOnAxis(ap=srow[:, 0:1], axis=0),
                        in_=flush[:], in_offset=None, compute_op=ALU.add)

            # ================ phase 1: layer-1 projections
            for name, s, d in RELS:
                load_w(f'Wl_{name}', FEAT[s], H1)
                load_w(f'Wr_{name}', FEAT[d], H1)
            for t, l_rels, r_rels in (
                ('news', ['ne', 'nt', 'nk'], ['en', 'tn', 'kn']),
                ('entities', ['en', 'ee'], ['ne', 'ee']),
                ('kg', ['kn'], ['nk']),
                ('topic', ['tn'], ['nt']),
            ):
                wn = [f'Wl_{r}' for r in l_rels] + [f'Wr_{r}' for r in r_rels]
                tabs = [intern[f'xl_{r}'] for r in l_rels] + \
                       [intern[f'xr_{r}'] for r in r_rels]
                project(inp['xT_' + t][:], wn, tabs, NPAD[t], FEAT[t], H1)

            # zero accumulators (8 tiles per iteration)
            for name, s, d in RELS:
                n = intern[f'ou1_{name}'].shape[0]
                with tc.For_i(0, n, 8 * P) as z0:
                    for kk in range(8):
                        nc.sync.dma_start(intern[f'ou1_{name}'][ds(z0 + kk * P, P)],
                                          zero_t[:, 0:H1 + 1])
            for name, s in L2_RELS:
                n = intern[f'ou2_{name}'].shape[0]
                with tc.For_i(0, n, 8 * P) as z0:
                    for kk in range(8):
                        nc.sync.dma_start(intern[f'ou2_{name}'][ds(z0 + kk * P, P)],
                                          zero_t[:, 0:H2 + 1])

            # ================ layer-1 edges
            for name, s, d in RELS:
                edge_phase(spec['l1'][name], H1,
                           intern[f'xl_{name}'], intern[f'xr_{name}'],
                           intern[f'ou1_{name}'], cst['att_' + name],
                           inp[f'src1_{name}'][:], inp[f'xri1_{name}'][:],
                           inp[f'slot1_{name}'][:], inp[f'rows1_{name}'][:])

            # ================ layer-1 normalize -> x2T
            def normalize(ou_tabs, n_rows, H, B_in, x2T_tab, final_lin=False):
                Bt = cpool.tile([P, H], F32, tag=B_in + "_t")
                nc.sync.dma_start(Bt[:], inp[B_in][:])
                with tc.For_i(0, n_rows, P) as r0:
                    agg = work.tile([P, H], F32, tag="agg")
                    for i, ou_t in enumerate(ou_tabs):
                        ou = work.tile([P, H + 1], F32, tag="ou")
                        nc.sync.dma_start(ou[:], ou_t[ds(r0, P)])
                        den = work.tile([P, 1], F32, tag="den")
                        nc.vector.tensor_scalar_add(den[:], ou[:, H:H + 1], 1e-38)
                        rec = work.tile([P, 1], F32, tag="rec")
                        nc.vector.reciprocal(rec[:], den[:])
                        if i == 0:
                            nc.vector.tensor_scalar_mul(agg[:], ou[:, 0:H], rec[:, 0:1])
                        else:
                            tmp = work.tile([P, H], F32, tag="tmp")
                            nc.vector.tensor_scalar_mul(tmp[:], ou[:, 0:H], rec[:, 0:1])
                            nc.vector.tensor_tensor(out=agg[:], in0=agg[:],
                                                    in1=tmp[:], op=ALU.add)
                    nc.vector.tensor_tensor(out=agg[:], in0=agg[:], in1=Bt[:], op=ALU.add)
                    nc.scalar.activation(agg[:], agg[:], AF.Relu)
                    if final_lin:
                        tp = psB.tile([P, P], F32, tag="tp")
                        nc.tensor.transpose(out=tp[:], in_=agg[:], identity=ident[:])
                        tps = work.tile([P, P], F32, tag="tps")
                        nc.vector.tensor_copy(tps[:], tp[:])
                        ops = psC.tile([P, 2], F32, tag="ops")
                        nc.tensor.matmul(ops[:], lhsT=tps[:], rhs=cst['lin_w'][:],
                                         start=True, stop=True)
                        ot2 = work.tile([P, 2], F32, tag="ot2")
                        nc.vector.tensor_tensor(out=ot2[:], in0=ops[:],
                                                in1=cst['lin_b'][:], op=ALU.add)
                        nc.sync.dma_start(out[ds(r0, P)], ot2[:])
                    else:
                        for kc in range(H // P):
                            tp = psB.tile([P, P], F32, tag="tp")
                            nc.tensor.transpose(out=tp[:], in_=agg[:, ds(kc * P, P)],
                                                identity=ident[:])
                            tps = work.tile([P, P], F32, tag="tps")
                            nc.vector.tensor_copy(tps[:], tp[:])
                            nc.sync.dma_start(
                                x2T_tab[kc * P:(kc + 1) * P, ds(r0, P)], tps[:])

            for t, rels_in in (('entities', ['ne', 'ee']),
                               ('news', ['en', 'tn', 'kn']),
                               ('kg', ['nk']), ('topic', ['nt'])):
                normalize([intern[f'ou1_{r}'] for r in rels_in], NPAD[t], H1,
                          'B1_' + t, intern['x2T_' + t])

            # ================ layer-2 projections
            for name, s in L2_RELS:
                load_w(f'W2l_{name}', H1, H2)
                load_w(f'W2r_{name}', H1, H2)
                project(intern['x2T_' + s][:], [f'W2l_{name}'],
                        [intern[f'xl2_{name}']], NPAD[s], H1, H2)
                project(intern['x2T_news'][:], [f'W2r_{name}'],
                        [intern[f'xr2_{name}']], NPAD['news'], H1, H2)

            # ================ layer-2 edges
            for name, s in L2_RELS:
                edge_phase(spec['l2'][name], H2,
                           intern[f'xl2_{name}'], intern[f'xr2_{name}'],
                           intern[f'ou2_{name}'], cst['att2_' + name],
                           inp[f'src2_{name}'][:], inp[f'xri2_{name}'][:],
                           inp[f'slot2_{name}'][:], inp[f'rows2_{name}'][:])

            # ================ layer-2 normalize + final linear
            lw = cpool.tile([P, 2], F32, tag="lin_w")
            nc.sync.dma_start(lw[:], inp['lin_w'][:])
            cst['lin_w'] = lw
            lb = cpool.tile([P, 2], F32, tag="lin_b")
            nc.sync.dma_start(lb[:], inp['lin_b'][:])
            cst['lin_b'] = lb
            normalize([intern[f'ou2_{nm}'] for nm, _ in L2_RELS], NEWS_SLICE,
                      H2, 'B2_news', None, final_lin=True)

    nc.compile()
    return nc


_CACHE = {}


def kernel(**inputs):
    per_core, spec = host_prep(inputs)
    key = (tuple(sorted(spec['l1'].items())), tuple(sorted(spec['l2'].items())))
    if key not in _CACHE:
        _CACHE[key] = build_kernel(spec)
    nc = _CACHE[key]
    res = run_bass_kernel_spmd(nc, per_core, core_ids=list(range(N_CORES)))
    full = np.concatenate([res.results[k]["out"] for k in range(N_CORES)], 0)
    return full[:NNODE['news']]


# revision 7
# speedup vs baseline: 1.2055x; 1.2055x over previous
"""Trainium2 Bass kernel for the KG hetero GNN (2-layer GATv2, 7 relations).

Sharding: layer 1 is computed fully on every core (v0); layer 2 only needs
relations into 'news' (the reference returns x['news'] @ lin_w, so x3 of the
other node types is dead) and is sharded by news row range across 8 cores.

Edge aggregation: edges sorted by dst, packed into "macros" of C*128 edge
slots with <=127 distinct dsts (slot 127 reserved for padding/trash).
Per chunk: indirect-gather xl[src], xr[dst] rows; t = lrelu(xl+xr);
e = sum(t*att); w = exp(e)  (the segment-max shift of softmax is skipped:
|e| stays orders of magnitude below the fp32 exp overflow threshold, and
the normalized ratio is identical up to rounding); a one-hot matmul
(lhsT[edge,slot] = w * (slot==dst_slot)) accumulates [slots, H+1] into PSUM
with column H accumulating w itself (softmax denominator); each macro is
flushed with an indirect scatter-add (CCE add) into a per-relation
accumulator table [rows, H+1].  A normalize pass divides by the
denominator, sums relations, adds bias, applies relu, and writes the
result transposed (x2T) so the next layer's projections need no on-device
transposes.
"""

import sys

sys.path.insert(0, "/opt/trn_rl_repo")

import numpy as np

import concourse.bass as bass
import concourse.mybir as mybir
import concourse.tile as tile
from concourse import bacc
from concourse.bass import ds
from concourse.bass_utils import run_bass_kernel_spmd
from concourse.masks import make_identity

AF = mybir.ActivationFunctionType
ALU = mybir.AluOpType
F32 = mybir.dt.float32
I32 = mybir.dt.int32
P = 128

N_CORES = 8
RELS = [('ne', 'news', 'entities'), ('en', 'entities', 'news'),
        ('nt', 'news', 'topic'), ('tn', 'topic', 'news'),
        ('nk', 'news', 'kg'), ('kn', 'kg', 'news'),
        ('ee', 'entities', 'entities')]
L2_RELS = [('en', 'entities'), ('tn', 'topic'), ('kn', 'kg')]
FEAT = {'news': 768, 'entities': 256, 'topic': 128, 'kg': 256}
NNODE = {'news': 100000, 'entities': 200000, 'topic': 500, 'kg': 200000}
NPAD = {'news': 100352, 'entities': 200704, 'topic': 512, 'kg': 200704}
H1, H2 = 256, 128
NEWS_SLICE = NPAD['news'] // N_CORES        # 12544
C = 2                                        # chunks per macro
TRASH = 127                                  # reserved slot


# ---------------------------------------------------------------- host prep

def _build_macros(src, dst, trash_row):
    """Sort edges by dst; pack into macros of C*128 edge slots with <=127
    distinct dsts.  Returns [nmac,C,128] src/dst/slot and [nmac,128] rows."""
    order = np.argsort(dst, kind='stable')
    src, dst = np.asarray(src)[order], np.asarray(dst)[order]
    E = len(src)
    cap = C * P
    src_m, dst_m, slot_m, rows_m = [], [], [], []
    i = 0
    while i < E:
        j = min(i + cap, E)
        d = dst[i:j]
        uniq = np.unique(d)
        if len(uniq) > TRASH:
            cut = int(np.searchsorted(d, uniq[TRASH]))
            j = i + cut
            d = dst[i:j]
            uniq = np.unique(d)
        ns = j - i
        s_pad = np.zeros(cap, np.int64)
        s_pad[:ns] = src[i:j]
        d_pad = np.zeros(cap, np.int64)
        d_pad[:ns] = d
        sl_pad = np.full(cap, TRASH, np.int64)
        sl_pad[:ns] = np.searchsorted(uniq, d)
        rows = np.full(P, trash_row, np.int64)
        rows[:len(uniq)] = uniq
        src_m.append(s_pad)
        dst_m.append(d_pad)
        slot_m.append(sl_pad)
        rows_m.append(rows)
        i = j
    nmac = len(src_m)
    out = (np.array(src_m, np.int32).reshape(nmac, C, P),
           np.array(dst_m, np.int32).reshape(nmac, C, P),
           np.array(slot_m, np.float32).reshape(nmac, C, P),
           np.array(rows_m, np.int32).reshape(nmac, P))
    # interleave macro order so consecutive flushes touch different dst
    # regions (scatter-add descriptors of in-flight DMAs must not collide)
    if nmac > 16:
        stride = max(1, nmac // 16)
        perm = np.concatenate([np.arange(k, nmac, stride) for k in range(stride)])
        out = tuple(a[perm] for a in out)
    return out


def _pad_macros(arrs, nmax, trash_row):
    """Pad macro arrays to nmax macros (pad macros are all-trash no-ops)."""
    sm, dm, slm, rm = arrs
    n = sm.shape[0]
    if n == nmax:
        return arrs
    sm2 = np.zeros((nmax, C, P), np.int32)
    dm2 = np.zeros((nmax, C, P), np.int32)
    slm2 = np.full((nmax, C, P), TRASH, np.float32)
    rm2 = np.full((nmax, P), trash_row, np.int32)
    sm2[:n], dm2[:n], slm2[:n], rm2[:n] = sm, dm, slm, rm
    return sm2, dm2, slm2, rm2


def host_prep(inputs):
    params = inputs['params']
    x = {t: np.asarray(inputs['x_' + t], np.float32) for t in FEAT}

    full_edges = {}
    for name, s, d in RELS:
        e = np.asarray(inputs['e_' + name])
        n_sl = min(NNODE[s], NNODE[d])
        loops = np.arange(n_sl, dtype=np.int64)
        full_edges[name] = (np.concatenate([e[0].astype(np.int64), loops]),
                            np.concatenate([e[1].astype(np.int64), loops]))

    l1p = params['layers'][0]
    l2p = params['layers'][1]
    spec = {'l1': {}, 'l2': {}}
    common = {}

    for t in FEAT:
        xt = np.zeros((FEAT[t], NPAD[t]), np.float32)
        xt[:, :NNODE[t]] = np.asarray(x[t]).T
        common['xT_' + t] = xt

    btype = {t: np.zeros(H1, np.float32) for t in FEAT}
    sharded_l1 = {}
    for name, s, d in RELS:
        lp = l1p[name]
        common[f'Wl_{name}'] = np.asarray(lp['Wl'], np.float32)
        common[f'Wr_{name}'] = np.asarray(lp['Wr'], np.float32)
        common[f'att_{name}'] = np.asarray(lp['att'], np.float32)[None, :]
        btype[d] = btype[d] + np.asarray(lp['b'], np.float32)
        src, dst = full_edges[name]
        if d == 'news':
            # only this core's news slice of x2 is ever consumed downstream,
            # so news-dst layer-1 edges can be sharded per core (host-only)
            data = []
            for k in range(N_CORES):
                lo = k * NEWS_SLICE
                m = (dst >= lo) & (dst < lo + NEWS_SLICE)
                data.append(_build_macros(src[m], dst[m] - lo + lo, NPAD[d]))
            nmax = ((max(a[0].shape[0] for a in data) + 3) // 4) * 4
            sharded_l1[name] = (data, nmax)
            spec['l1'][name] = nmax
        else:
            arrs = _build_macros(src, dst, NPAD[d])
            nmax = ((arrs[0].shape[0] + 3) // 4) * 4
            sm, dm, slm, rm = _pad_macros(arrs, nmax, NPAD[d])
            common[f'src1_{name}'] = sm
            common[f'xri1_{name}'] = dm
            common[f'slot1_{name}'] = slm
            common[f'rows1_{name}'] = rm
            spec['l1'][name] = nmax
    for t in FEAT:
        common[f'B1_{t}'] = np.broadcast_to(btype[t], (P, H1)).copy()

    b2 = np.zeros(H2, np.float32)
    for name, s in L2_RELS:
        lp = l2p[name]
        b2 += np.asarray(lp['b'], np.float32)
        common[f'W2l_{name}'] = np.asarray(lp['Wl'], np.float32)
        common[f'W2r_{name}'] = np.asarray(lp['Wr'], np.float32)
        common[f'att2_{name}'] = np.asarray(lp['att'], np.float32)[None, :]
    common['B2_news'] = np.broadcast_to(b2, (P, H2)).copy()
    common['lin_w'] = np.asarray(params['lin_w'], np.float32)
    common['lin_b'] = np.broadcast_to(np.asarray(params['lin_b'], np.float32),
                                      (P, 2)).copy()

    per_core = [dict(common) for _ in range(N_CORES)]
    for name, (data, nmax) in sharded_l1.items():
        for k in range(N_CORES):
            sm2, dm2, slm2, rm2 = _pad_macros(data[k], nmax, NPAD['news'])
            pc = per_core[k]
            pc[f'src1_{name}'] = sm2
            pc[f'xri1_{name}'] = dm2
            pc[f'slot1_{name}'] = slm2
            pc[f'rows1_{name}'] = rm2
    for name, s in L2_RELS:
        src, dst = full_edges[name]
        data = []
        for k in range(N_CORES):
            lo = k * NEWS_SLICE
            m = (dst >= lo) & (dst < lo + NEWS_SLICE)
            sm, dm, slm, rm = _build_macros(src[m], dst[m] - lo, NEWS_SLICE)
            data.append((sm, dm + lo, slm, rm))   # xr idx global, rows local
        nmax = ((max(a[0].shape[0] for a in data) + 3) // 4) * 4
        spec['l2'][name] = nmax
        for k in range(N_CORES):
            sm2, dm2, slm2, rm2 = _pad_macros(data[k], nmax, NEWS_SLICE)
            pc = per_core[k]
            pc[f'src2_{name}'] = sm2
            pc[f'xri2_{name}'] = dm2
            pc[f'slot2_{name}'] = slm2
            pc[f'rows2_{name}'] = rm2
    return per_core, spec


# ---------------------------------------------------------------- device

def build_kernel(spec):
    nc = bacc.Bacc(None, target_bir_lowering=False, debug=False,
                   num_devices=N_CORES)
    inp = {}

    def ext_in(nm, shape, dt=F32):
        inp[nm] = nc.dram_tensor(nm, list(shape), dt, kind="ExternalInput")

    for t in FEAT:
        ext_in('xT_' + t, [FEAT[t], NPAD[t]])
        ext_in('B1_' + t, [P, H1])
    for name, s, d in RELS:
        ext_in(f'Wl_{name}', [FEAT[s], H1])
        ext_in(f'Wr_{name}', [FEAT[d], H1])
        ext_in(f'att_{name}', [1, H1])
        nm = spec['l1'][name]
        ext_in(f'src1_{name}', [nm, C, P], I32)
        ext_in(f'xri1_{name}', [nm, C, P], I32)
        ext_in(f'slot1_{name}', [nm, C, P])
        ext_in(f'rows1_{name}', [nm, P], I32)
    for name, s in L2_RELS:
        ext_in(f'W2l_{name}', [H1, H2])
        ext_in(f'W2r_{name}', [H1, H2])
        ext_in(f'att2_{name}', [1, H2])
        nm = spec['l2'][name]
        ext_in(f'src2_{name}', [nm, C, P], I32)
        ext_in(f'xri2_{name}', [nm, C, P], I32)
        ext_in(f'slot2_{name}', [nm, C, P])
        ext_in(f'rows2_{name}', [nm, P], I32)
    ext_in('B2_news', [P, H2])
    ext_in('lin_w', [H2, 2])
    ext_in('lin_b', [P, 2])
    out = nc.dram_tensor("out", [NEWS_SLICE, 2], F32, kind="ExternalOutput")

    intern = {}

    def itab(nm, shape):
        intern[nm] = nc.dram_tensor(nm, list(shape), F32, kind="Internal")

    def r1024(n):
        return ((n + 1023) // 1024) * 1024

    for name, s, d in RELS:
        itab(f'xl_{name}', [NPAD[s], H1])
        itab(f'xr_{name}', [NPAD[d], H1])
        itab(f'ou1_{name}', [r1024(NPAD[d] + P), H1 + 1])
    for t in FEAT:
        itab(f'x2T_{t}', [H1, NPAD[t]])
    for name, s in L2_RELS:
        itab(f'xl2_{name}', [NPAD[s], H2])
        itab(f'xr2_{name}', [NPAD['news'], H2])
        itab(f'ou2_{name}', [r1024(NEWS_SLICE + P), H2 + 1])

    with tile.TileContext(nc) as tc:
        with (
            tc.tile_pool(name="const", bufs=1) as cpool,
            tc.tile_pool(name="work", bufs=4) as work,
            tc.tile_pool(name="psA", bufs=2, space="PSUM") as psA,
            tc.tile_pool(name="psB", bufs=2, space="PSUM") as psB,
            tc.tile_pool(name="psC", bufs=1, space="PSUM") as psC,
        ):
            cst = {}
            iota = cpool.tile([P, P], I32)
            nc.gpsimd.iota(iota[:], pattern=[[1, P]], base=0, channel_multiplier=0)
            iota_f = cpool.tile([P, P], F32)
            nc.vector.tensor_copy(iota_f[:], iota[:])
            ones_col = cpool.tile([1, P], F32)
            nc.vector.memset(ones_col[:], 1.0)
            zero_t = cpool.tile([P, H1 + 1], F32)
            nc.vector.memset(zero_t[:], 0.0)
            ident = cpool.tile([P, P], F32)
            make_identity(nc, ident[:])

            def bcast_row(nm, width):
                row = cpool.tile([1, width], F32, tag=nm + "_r")
                nc.sync.dma_start(row[:], inp[nm][:])
                ps = psC.tile([P, width], F32, tag="bc")
                nc.tensor.matmul(ps[:], lhsT=ones_col[:], rhs=row[:],
                                 start=True, stop=True)
                tl = cpool.tile([P, width], F32, tag=nm + "_b")
                nc.scalar.copy(tl[:], ps[:])
                return tl

            for name, s, d in RELS:
                cst['att_' + name] = bcast_row(f'att_{name}', H1)
            for name, s in L2_RELS:
                cst['att2_' + name] = bcast_row(f'att2_{name}', H2)

            def load_w(nm, F, Hout):
                wt = cpool.tile([P, F // P, Hout], F32, tag=nm)
                nc.sync.dma_start(wt[:], inp[nm][:].rearrange("(k p) h -> p k h", p=P))
                cst[nm] = wt

            # ------------ projections helper
            def project(xT_ap, w_names, out_tables, n_rows, F, Hout):
                kch = F // P
                with tc.For_i(0, n_rows, P) as r0:
                    xt = work.tile([P, kch, P], F32, tag="xt")
                    nc.sync.dma_start(
                        xt[:],
                        xT_ap.rearrange("(k p) n -> p k n", p=P)[:, :, ds(r0, P)])
                    for i, wn in enumerate(w_names):
                        accw = psA.tile([P, Hout], F32, tag="accw")
                        for kc in range(kch):
                            nc.tensor.matmul(accw[:], lhsT=xt[:, kc, :],
                                             rhs=cst[wn][:, kc, :],
                                             start=(kc == 0), stop=(kc == kch - 1))
                        ot = work.tile([P, Hout], F32, tag="ot")
                        nc.scalar.copy(ot[:], accw[:])
                        nc.sync.dma_start(out_tables[i][ds(r0, P)], ot[:])

            # ------------ edge phase helper
            def edge_phase(nmac, H, xl_tab, xr_tab, ou_tab, att_b,
                           srcA, xriA, slotA, rowsA, UNROLL=4):
                n_iter = (nmac + UNROLL - 1) // UNROLL
                with tc.For_i(0, n_iter * UNROLL, UNROLL) as m0:
                  for uu in range(UNROLL):
                    m = m0 + uu
                    sidx = work.tile([P, C], I32, tag="sidx")
                    ridx = work.tile([P, C], I32, tag="ridx")
                    slot = work.tile([P, C], F32, tag="slot")
                    srow = work.tile([P, 1], I32, tag="srow")
                    nc.sync.dma_start(sidx[:], srcA[ds(m, 1)].rearrange("o c p -> (o p) c"))
                    nc.sync.dma_start(ridx[:], xriA[ds(m, 1)].rearrange("o c p -> (o p) c"))
                    nc.sync.dma_start(slot[:], slotA[ds(m, 1)].rearrange("o c p -> (o p) c"))
                    nc.sync.dma_start(srow[:], rowsA[ds(m, 1)].rearrange("o (p u) -> (o p) u", u=1))

                    g_xl = work.tile([P, C, H + 1], F32, tag="gxl")
                    g_xr = work.tile([P, C, H], F32, tag="gxr")
                    for c in range(C):
                        nc.gpsimd.indirect_dma_start(
                            out=g_xl[:, c, 0:H], out_offset=None, in_=xl_tab[:],
                            in_offset=bass.IndirectOffsetOnAxis(ap=sidx[:, c:c + 1], axis=0))
                        nc.gpsimd.indirect_dma_start(
                            out=g_xr[:, c, :], out_offset=None, in_=xr_tab[:],
                            in_offset=bass.IndirectOffsetOnAxis(ap=ridx[:, c:c + 1], axis=0))
                    nc.vector.memset(g_xl[:, :, H:H + 1], 1.0)

                    t = work.tile([P, C, H], F32, tag="t")
                    nc.vector.tensor_tensor(out=t[:], in0=g_xl[:, :, 0:H],
                                            in1=g_xr[:], op=ALU.add)
                    nc.scalar.activation(t[:], t[:], AF.Prelu, alpha=0.2)
                    nc.vector.tensor_tensor(
                        out=t[:], in0=t[:],
                        in1=att_b[:].rearrange("p (u d) -> p u d", u=1).to_broadcast([P, C, H]),
                        op=ALU.mult)
                    e = work.tile([P, C], F32, tag="e")
                    nc.vector.tensor_reduce(out=e[:], in_=t[:],
                                            axis=mybir.AxisListType.X, op=ALU.add)
                    w = work.tile([P, C], F32, tag="w")
                    nc.scalar.activation(w[:], e[:], AF.Exp)

                    acc = psA.tile([P, H + 1], F32, tag="acc")
                    for c in range(C):
                        S = work.tile([P, P], F32, tag="S")
                        nc.vector.tensor_scalar(out=S[:], in0=iota_f[:],
                                                scalar1=slot[:, c:c + 1],
                                                scalar2=w[:, c:c + 1],
                                                op0=ALU.is_equal, op1=ALU.mult)
                        nc.tensor.matmul(acc[:], lhsT=S[:], rhs=g_xl[:, c, :],
                                         start=(c == 0), stop=(c == C - 1))
                    flush = work.tile([P, H + 1], F32, tag="flush")
                    nc.scalar.copy(flush[:], acc[:])
                    nc.gpsimd.indirect_dma_start(
                        out=ou_tab[:],
                        out_offset=bass.IndirectOffsetOnAxis(ap=srow[:, 0:1], axis=0),
                        in_=flush[:], in_offset=None, compute_op=ALU.add)

            # ================ phase 1: layer-1 projections
            for name, s, d in RELS:
                load_w(f'Wl_{name}', FEAT[s], H1)
                load_w(f'Wr_{name}', FEAT[d], H1)
            for t, l_rels, r_rels in (
                ('news', ['ne', 'nt', 'nk'], ['en', 'tn', 'kn']),
                ('entities', ['en', 'ee'], ['ne', 'ee']),
                ('kg', ['kn'], ['nk']),
                ('topic', ['tn'], ['nt']),
            ):
                wn = [f'Wl_{r}' for r in l_rels] + [f'Wr_{r}' for r in r_rels]
                tabs = [intern[f'xl_{r}'] for r in l_rels] + \
                       [intern[f'xr_{r}'] for r in r_rels]
                project(inp['xT_' + t][:], wn, tabs, NPAD[t], FEAT[t], H1)

            # zero accumulators (8 tiles per iteration)
            for name, s, d in RELS:
                n = intern[f'ou1_{name}'].shape[0]
                with tc.For_i(0, n, 8 * P) as z0:
                    for kk in range(8):
                        nc.sync.dma_start(intern[f'ou1_{name}'][ds(z0 + kk * P, P)],
                                          zero_t[:, 0:H1 + 1])
            for name, s in L2_RELS:
                n = intern[f'ou2_{name}'].shape[0]
                with tc.For_i(0, n, 8 * P) as z0:
                    for kk in range(8):
                        nc.sync.dma_start(intern[f'ou2_{name}'][ds(z0 + kk * P, P)],
                                          zero_t[:, 0:H2 + 1])

            # ================ layer-1 edges
            for name, s, d in RELS:
                edge_phase(spec['l1'][name], H1,
                           intern[f'xl_{name}'], intern[f'xr_{name}'],
                           intern[f'ou1_{name}'], cst['att_' + name],
                           inp[f'src1_{name}'][:], inp[f'xri1_{name}'][:],
                           inp[f'slot1_{name}'][:], inp[f'rows1_{name}'][:])

            # ================ layer-1 normalize -> x2T
            def normalize(ou_tabs, n_rows, H, B_in, x2T_tab, final_lin=False):
                Bt = cpool.tile([P, H], F32, tag=B_in + "_t")
                nc.sync.dma_start(Bt[:], inp[B_in][:])
                with tc.For_i(0, n_rows, P) as r0:
                    agg = work.tile([P, H], F32, tag="agg")
                    for i, ou_t in enumerate(ou_tabs):
                        ou = work.tile([P, H + 1], F32, tag="ou")
                        nc.sync.dma_start(ou[:], ou_t[ds(r0, P)])
                        den = work.tile([P, 1], F32, tag="den")
                        nc.vector.tensor_scalar_add(den[:], ou[:, H:H + 1], 1e-38)
                        rec = work.tile([P, 1], F32, tag="rec")
                        nc.vector.reciprocal(rec[:], den[:])
                        if i == 0:
                            nc.vector.tensor_scalar_mul(agg[:], ou[:, 0:H], rec[:, 0:1])
                        else:
                            tmp = work.tile([P, H], F32, tag="tmp")
                            nc.vector.tensor_scalar_mul(tmp[:], ou[:, 0:H], rec[:, 0:1])
                            nc.vector.tensor_tensor(out=agg[:], in0=agg[:],
                                                    in1=tmp[:], op=ALU.add)
                    nc.vector.tensor_tensor(out=agg[:], in0=agg[:], in1=Bt[:], op=ALU.add)
                    nc.scalar.activation(agg[:], agg[:], AF.Relu)
                    if final_lin:
                        tp = psB.tile([P, P], F32, tag="tp")
                        nc.tensor.transpose(out=tp[:], in_=agg[:], identity=ident[:])
                        tps = work.tile([P, P], F32, tag="tps")
                        nc.vector.tensor_copy(tps[:], tp[:])
                        ops = psC.tile([P, 2], F32, tag="ops")
                        nc.tensor.matmul(ops[:], lhsT=tps[:], rhs=cst['lin_w'][:],
                                         start=True, stop=True)
                        ot2 = work.tile([P, 2], F32, tag="ot2")
                        nc.vector.tensor_tensor(out=ot2[:], in0=ops[:],
                                                in1=cst['lin_b'][:], op=ALU.add)
                        nc.sync.dma_start(out[ds(r0, P)], ot2[:])
                    else:
                        for kc in range(H // P):
                            tp = psB.tile([P, P], F32, tag="tp")
                            nc.tensor.transpose(out=tp[:], in_=agg[:, ds(kc * P, P)],
                                                identity=ident[:])
                            tps = work.tile([P, P], F32, tag="tps")
                            nc.vector.tensor_copy(tps[:], tp[:])
                            nc.sync.dma_start(
                                x2T_tab[kc * P:(kc + 1) * P, ds(r0, P)], tps[:])

            for t, rels_in in (('entities', ['ne', 'ee']),
                               ('news', ['en', 'tn', 'kn']),
                               ('kg', ['nk']), ('topic', ['nt'])):
                normalize([intern[f'ou1_{r}'] for r in rels_in], NPAD[t], H1,
                          'B1_' + t, intern['x2T_' + t])

            # ================ layer-2 projections
            for name, s in L2_RELS:
                load_w(f'W2l_{name}', H1, H2)
                load_w(f'W2r_{name}', H1, H2)
                project(intern['x2T_' + s][:], [f'W2l_{name}'],
                        [intern[f'xl2_{name}']], NPAD[s], H1, H2)
                project(intern['x2T_news'][:], [f'W2r_{name}'],
                        [intern[f'xr2_{name}']], NPAD['news'], H1, H2)

            # ================ layer-2 edges
            for name, s in L2_RELS:
                edge_phase(spec['l2'][name], H2,
                           intern[f'xl2_{name}'], intern[f'xr2_{name}'],
                           intern[f'ou2_{name}'], cst['att2_' + name],
                           inp[f'src2_{name}'][:], inp[f'xri2_{name}'][:],
                           inp[f'slot2_{name}'][:], inp[f'rows2_{name}'][:])

            # ================ layer-2 normalize + final linear
            lw = cpool.tile([P, 2], F32, tag="lin_w")
            nc.sync.dma_start(lw[:], inp['lin_w'][:])
            cst['lin_w'] = lw
            lb = cpool.tile([P, 2], F32, tag="lin_b")
            nc.sync.dma_start(lb[:], inp['lin_b'][:])
            cst['lin_b'] = lb
            normalize([intern[f'ou2_{nm}'] for nm, _ in L2_RELS], NEWS_SLICE,
                      H2, 'B2_news', None, final_lin=True)

    nc.compile()
    return nc


_CACHE = {}


def kernel(**inputs):
    per_core, spec = host_prep(inputs)
    key = (tuple(sorted(spec['l1'].items())), tuple(sorted(spec['l2'].items())))
    if key not in _CACHE:
        _CACHE[key] = build_kernel(spec)
    nc = _CACHE[key]
    res = run_bass_kernel_spmd(nc, per_core, core_ids=list(range(N_CORES)))
    full = np.concatenate([res.results[k]["out"] for k in range(N_CORES)], 0)
    return full[:NNODE['news']]
